# revision 2
# baseline (speedup 1.0000x reference)
"""Trainium2 Bass kernel for nn_CPE_47364899340506 (submanifold sparse 3D conv).

Reference semantics: coords quantized from depth onto a 65^3 voxel grid, a
global voxel->point-index map (max-index dedup), then for each of 27 kernel
offsets gather active-neighbor features and GEMM with the per-offset
[256, 256] weight, accumulating over offsets.

Strategy:
  Host (numpy, cheap integer work): replicate the coords / voxel-map / neighbor
  index computation bit-exactly, shard the 65552 points across 8 cores by image
  row-band (spatially local), and build per-core compact "winner" feature
  tables (~9K rows) so gather indices fit int16.

  Device (per core, SPMD): for each 128-point tile, one dma_gather(transpose=
  True) pulls all 27 taps' neighbor features from the compact table directly in
  [ci, pt] (transposed) layout, then 54 back-to-back matmuls (27 taps x 2
  ci-chunks, fp16 in / fp32 psum accumulate) produce the [128, 256] output
  tile.  Invalid (out-of-bounds / empty-voxel) taps gather a zero row.
"""
import itertools
from contextlib import ExitStack

import numpy as np

BND = 64
G = BND + 1
B, H, W, C = 16, 64, 64, 256
HW = H * W
N = B * (HW + 1)              # 65552
NCORES = 8
NLOC = N // NCORES            # 8194
P = 128
NT = (NLOC + P - 1) // P      # 65 tiles (last has 2 live rows)
TAPS = 27
CHUNKS = 2                    # ci chunks of 128
NIDX = TAPS * P               # 3456 gather indices per point tile
S = NIDX // 16
OFFSETS = np.array(list(itertools.product([-1, 0, 1], repeat=3)), dtype=np.int32)

_COMPILED = {}                # (u_pad,) -> nc


# ---------------------------------------------------------------- host prep --

def _compute_coords(depth):
    ah = np.arange(H, dtype=np.float32) / np.float32(H - 1)
    aw = np.arange(W, dtype=np.float32) / np.float32(W - 1)
    y, x = np.meshgrid(ah, aw, indexing="ij")
    zmin = depth.min(axis=(1, 2), keepdims=True)
    zmax = depth.max(axis=(1, 2), keepdims=True)
    z = (depth - zmin) / (zmax - zmin + np.float32(1e-8))
    bx = np.broadcast_to(x, (B, H, W)).astype(np.float32)
    by = np.broadcast_to(y, (B, H, W)).astype(np.float32)
    coords = np.stack([bx, by, z], axis=-1)
    coord = coords.reshape(B, HW, 3)
    coord = np.clip(np.round(coord * np.float32(BND)), 0, BND).astype(np.int32)
    cls = np.zeros((B, 1, 3), dtype=np.int32)
    return np.concatenate([cls, coord], axis=1).reshape(-1, 3)


def _compute_nid_valid(coord):
    lin = (coord[:, 0] * G + coord[:, 1]) * G + coord[:, 2]
    idx_map = np.full((G * G * G,), -1, dtype=np.int32)
    np.maximum.at(idx_map, lin, np.arange(N, dtype=np.int32))
    nb = coord[None, :, :] + OFFSETS[:, None, :]
    inb = np.all((nb >= 0) & (nb <= BND), axis=-1)
    nbc = np.clip(nb, 0, BND)
    nlin = (nbc[..., 0] * G + nbc[..., 1]) * G + nbc[..., 2]
    nid = idx_map[nlin]
    valid = inb & (nid >= 0)
    return nid, valid


def _core_point_assignment():
    idx = np.arange(N, dtype=np.int32)
    rel = idx % (HW + 1)
    batch = idx // (HW + 1)
    is_cls = rel == 0
    row = (rel - 1) // W
    band = np.where(is_cls, batch // 2, row // 8)
    return np.argsort(band, kind="stable").astype(np.int32).reshape(NCORES, NLOC)


def _build_core_tables(features, nid, valid, perm):
    per_core_w, per_core_loc, ucs = [], [], []
    for c in range(NCORES):
        nid_c = nid[:, perm[c]]
        val_c = valid[:, perm[c]]
        used = np.unique(nid_c[val_c])
        ucs.append(len(used))
        loc = np.full(nid_c.shape, -1, dtype=np.int32)
        loc[val_c] = np.searchsorted(used, nid_c[val_c]).astype(np.int32)
        per_core_w.append(used)
        per_core_loc.append(loc)
    u_pad = ((max(ucs) + 1 + 127) // 128) * 128
    assert u_pad <= 32767, f"winner table too large for int16: {u_pad}"
    sent = u_pad - 1
    xw = np.zeros((NCORES, u_pad, C), dtype=np.float16)
    idxw = np.zeros((NCORES, P, NT * S), dtype=np.int16)
    for c in range(NCORES):
        used = per_core_w[c]
        xw[c, : len(used)] = features[used].astype(np.float16)
        xw[c, sent] = 0
        loc = np.where(per_core_loc[c] < 0, sent, per_core_loc[c])   # [27, NLOC]
        pad = np.full((TAPS, NT * P - NLOC), sent, dtype=np.int32)
        locp = np.concatenate([loc, pad], axis=1).reshape(TAPS, NT, P)
        flat = locp.transpose(1, 0, 2).reshape(NT, NIDX)
        wrapped = flat.reshape(NT, S, 16).transpose(0, 2, 1)          # [NT,16,S]
        idxw[c] = np.tile(wrapped, (1, 8, 1)).transpose(1, 0, 2).reshape(P, NT * S)
    return xw, idxw, u_pad


def _build_weight_input(weight):
    w = weight.astype(np.float16).reshape(TAPS, CHUNKS, P, C)
    return np.ascontiguousarray(w.transpose(2, 0, 1, 3).reshape(P, TAPS * CHUNKS * C))


# ------------------------------------------------------------- device kernel --

def _build_bass(u_pad):
    import concourse.bacc as bacc
    import concourse.tile as tile
    from concourse import mybir

    F16, F32, I16 = mybir.dt.float16, mybir.dt.float32, mybir.dt.int16
    nc = bacc.Bacc("TRN2", target_bir_lowering=False, debug=False,
                   num_devices=NCORES)
    xw = nc.dram_tensor("xw", [u_pad, C], F16, kind="ExternalInput").ap()
    idx = nc.dram_tensor("idx", [P, NT * S], I16, kind="ExternalInput").ap()
    wts = nc.dram_tensor("wts", [P, TAPS * CHUNKS * C], F16, kind="ExternalInput").ap()
    out = nc.dram_tensor("out", [NLOC, C], F32, kind="ExternalOutput").ap()

    with tile.TileContext(nc) as tc, ExitStack() as ctx:
        const_pool = ctx.enter_context(tc.tile_pool(name="const", bufs=1))
        gpool = ctx.enter_context(tc.tile_pool(name="gather", bufs=3))
        pspool = ctx.enter_context(tc.tile_pool(name="psum", bufs=4, space="PSUM"))
        opool = ctx.enter_context(tc.tile_pool(name="outp", bufs=3))

        w_tile = const_pool.tile([P, TAPS * CHUNKS * C], F16, tag="wts")
        nc.sync.dma_start(out=w_tile[:], in_=wts[:])
        idx_tile = const_pool.tile([P, NT * S], I16, tag="idx")
        nc.sync.dma_start(out=idx_tile[:], in_=idx[:])

        for t in range(NT):
            g = gpool.tile([P, CHUNKS, NIDX], F16, tag="g")
            nc.gpsimd.dma_gather(
                out_ap=g[:, :, :],
                in_ap=xw[:, :],
                idxs_ap=idx_tile[:, t * S:(t + 1) * S],
                num_idxs=NIDX,
                num_idxs_reg=NIDX,
                elem_size=C,
                transpose=True,
                single_packet=False,
            )
            ps = pspool.tile([P, C], F32)
            i_mm = 0
            for k in range(TAPS):
                for cc in range(CHUNKS):
                    nc.tensor.matmul(
                        ps[:, :],
                        lhsT=g[:, cc, k * P:(k + 1) * P],
                        rhs=w_tile[:, (k * CHUNKS + cc) * C:(k * CHUNKS + cc + 1) * C],
                        start=(i_mm == 0),
                        stop=(i_mm == TAPS * CHUNKS - 1),
                    )
                    i_mm += 1
            o = opool.tile([P, C], F32)
            nc.vector.tensor_copy(o[:, :], ps[:, :])
            rows = min(P, NLOC - t * P)
            nc.sync.dma_start(out=out[t * P:t * P + rows, :], in_=o[:rows, :])
    nc.compile()
    return nc


# --------------------------------------------------------------- entry point --

def kernel(features, depth, weight):
    from concourse.bass_utils import run_bass_kernel_spmd

    features = np.asarray(features, dtype=np.float32)
    depth = np.asarray(depth, dtype=np.float32)
    weight = np.asarray(weight, dtype=np.float32)

    coord = _compute_coords(depth)
    nid, valid = _compute_nid_valid(coord)
    perm = _core_point_assignment()
    xw, idxw, u_pad = _build_core_tables(features, nid, valid, perm)
    w_dev = _build_weight_input(weight)

    if u_pad not in _COMPILED:
        _COMPILED[u_pad] = _build_bass(u_pad)
    nc = _COMPILED[u_pad]

    in_maps = [{"xw": xw[c], "idx": idxw[c], "wts": w_dev} for c in range(NCORES)]
    res = run_bass_kernel_spmd(nc, in_maps, list(range(NCORES)))

    out = np.empty((N, C), dtype=np.float32)
    for c in range(NCORES):
        out[perm[c]] = res.results[c]["out"]
    return out


# revision 3
# speedup vs baseline: 3.0107x; 3.0107x over previous
"""Trainium2 Bass kernel for nn_CPE_47364899340506 (submanifold sparse 3D conv).

Reference semantics: coords quantized from depth onto a 65^3 voxel grid, a
global voxel->point-index map (max-index dedup), then for each of 27 kernel
offsets gather active-neighbor features and GEMM with the per-offset
[256, 256] weight, accumulating over offsets.

Strategy (8 NeuronCores, SPMD, full inputs in / full output out):
  Host (numpy integer work, bit-exact replica of the reference index math):
    - shard the 65552 points by image row-band (8 rows per core), voxel-sorted
      within each core;
    - per core, per group of ~9 point-tiles, build a compact voxel-sorted
      "winner" feature table; because the table is voxel-sorted, the three
      dz in {-1,0,1} taps of any (point, direction) triple always hit
      CONSECUTIVE table rows, so each triple is served by ONE 1.5KB gather
      descriptor from a pattern-region table (valid/invalid combos encoded as
      contiguous 3-unit patterns: plain run / [0,X,0,X..] / [0,0,X_m,X_m+1]
      blocks).  This cuts DMA descriptors 3x vs per-tap gathering - the
      SDMA descriptor rate (~9 ns/descriptor aggregate) is the bottleneck.
  Device (per core): for each 128-point tile, one dma_gather(transpose=True)
  (1152 descriptors, 1.5KB each) lands all 27 taps' neighbor features in
  [ci, pt] layout; 54 back-to-back fp16 matmuls (27 taps x 2 ci-chunks)
  accumulate the [128, 256] output tile in fp32 PSUM.
"""
import itertools
from contextlib import ExitStack

import numpy as np

BND = 64
G = BND + 1
B, H, W, C = 16, 64, 64, 256
HW = H * W
N = B * (HW + 1)              # 65552
NCORES = 8
NLOC = N // NCORES            # 8194
P = 128
NT = (NLOC + P - 1) // P      # 65 tiles (last has 2 live rows)
TAPS = 27
CHUNKS = 2
TRI_NIDX = 9 * P              # one 3-tap descriptor per (point, direction)
TRI_S = TRI_NIDX // 16
TILE_GRP = 9                  # tiles per winner-table group
NGRP = (NT + TILE_GRP - 1) // TILE_GRP
OFFSETS = np.array(list(itertools.product([-1, 0, 1], repeat=3)), dtype=np.int32)

_COMPILED = {}


# ---------------------------------------------------------------- host prep --

def _compute_coords(depth):
    ah = np.arange(H, dtype=np.float32) / np.float32(H - 1)
    aw = np.arange(W, dtype=np.float32) / np.float32(W - 1)
    y, x = np.meshgrid(ah, aw, indexing="ij")
    zmin = depth.min(axis=(1, 2), keepdims=True)
    zmax = depth.max(axis=(1, 2), keepdims=True)
    z = (depth - zmin) / (zmax - zmin + np.float32(1e-8))
    bx = np.broadcast_to(x, (B, H, W)).astype(np.float32)
    by = np.broadcast_to(y, (B, H, W)).astype(np.float32)
    coords = np.stack([bx, by, z], axis=-1)
    coord = coords.reshape(B, HW, 3)
    coord = np.clip(np.round(coord * np.float32(BND)), 0, BND).astype(np.int32)
    cls = np.zeros((B, 1, 3), dtype=np.int32)
    return np.concatenate([cls, coord], axis=1).reshape(-1, 3)


def _compute_nid_valid(coord):
    lin = (coord[:, 0] * G + coord[:, 1]) * G + coord[:, 2]
    idx_map = np.full((G * G * G,), -1, dtype=np.int32)
    np.maximum.at(idx_map, lin, np.arange(N, dtype=np.int32))
    nb = coord[None, :, :] + OFFSETS[:, None, :]
    inb = np.all((nb >= 0) & (nb <= BND), axis=-1)
    nbc = np.clip(nb, 0, BND)
    nlin = (nbc[..., 0] * G + nbc[..., 1]) * G + nbc[..., 2]
    nid = idx_map[nlin]
    valid = inb & (nid >= 0)
    return nid, valid


def _core_point_assignment(coord):
    idx = np.arange(N, dtype=np.int32)
    rel = idx % (HW + 1)
    batch = idx // (HW + 1)
    is_cls = rel == 0
    row = (rel - 1) // W
    band = np.where(is_cls, batch // 2, row // 8)
    order = np.argsort(band, kind="stable").astype(np.int32)
    perm = order.reshape(NCORES, NLOC)
    voxkey = (coord[:, 1].astype(np.int64) * G + coord[:, 0]) * G + coord[:, 2]
    return np.stack([p[np.argsort(voxkey[p], kind="stable")] for p in perm])


def _units_total(rows):
    q2 = rows + 2
    q3 = q2 + 4 + 2 * (rows + 1) + 2 + 4   # [X_0,0,0,0] pad block at q3-4
    return q3 + 4 * (rows + 1) + 4, q2, q3


def _build_triple_tables(features, coord, nid, valid, perm):
    voxkey = (coord[:, 1].astype(np.int64) * G + coord[:, 0]) * G + coord[:, 2]
    per_core = []
    max_rows = 0
    for c in range(NCORES):
        pts_all = perm[c]
        groups = []
        for g in range(NGRP):
            t0, t1 = g * TILE_GRP, min((g + 1) * TILE_GRP, NT)
            p0, p1 = t0 * P, min(t1 * P, NLOC)
            pts = pts_all[p0:p1]
            nid_g = nid[:, pts]
            val_g = valid[:, pts]
            used = np.unique(nid_g[val_g])
            used = used[np.argsort(voxkey[used], kind="stable")]
            rows = len(used)
            max_rows = max(max_rows, rows)
            gid_order = np.argsort(used)
            gid_sorted = used[gid_order]

            def lookup(garr):
                return gid_order[np.searchsorted(gid_sorted, garr)]

            tot, q2, q3 = _units_total(rows)
            npts = p1 - p0
            units = np.zeros((9, npts), dtype=np.int64)
            for d in range(9):
                k0, k1, k2 = d * 3, d * 3 + 1, d * 3 + 2
                v1, v2, v3 = val_g[k0], val_g[k1], val_g[k2]
                r1 = np.where(v1, lookup(np.where(v1, nid_g[k0], used[0])), -1)
                r2 = np.where(v2, lookup(np.where(v2, nid_g[k1], used[0])), -1)
                r3 = np.where(v3, lookup(np.where(v3, nid_g[k2], used[0])), -1)
                combo = v1.astype(np.int64) * 4 + v2 * 2 + v3
                u = np.full(npts, q2, dtype=np.int64)          # (i,i,i)
                u = np.where(combo == 7, r1, u)                # (v,v,v)
                np.testing.assert_array_equal(r2[combo == 7], r1[combo == 7] + 1)
                np.testing.assert_array_equal(r3[combo == 7], r1[combo == 7] + 2)
                u = np.where(combo == 2, q2 + 4 + 2 * r2, u)   # (i,v,i)
                u = np.where(combo == 5, q2 + 5 + 2 * r1, u)   # (v,i,v)
                np.testing.assert_array_equal(r3[combo == 5], r1[combo == 5] + 1)
                u = np.where(combo == 1, q3 + 4 * r3, u)       # (i,i,v)
                u = np.where(combo == 3, q3 + 4 * r2 + 1, u)   # (i,v,v)
                np.testing.assert_array_equal(r3[combo == 3], r2[combo == 3] + 1)
                u = np.where(combo == 6, q3 + 4 * r1 + 2, u)   # (v,v,i)
                np.testing.assert_array_equal(r2[combo == 6], r1[combo == 6] + 1)
                u = np.where(combo == 4,                       # (v,i,i)
                             np.where(r1 == 0, q3 - 4, q3 + 4 * r1 - 1), u)
                units[d] = u
            groups.append((used, units))
        per_core.append(groups)
    u_tot_max, _, _ = _units_total(max_rows)
    u_sub = ((u_tot_max + 127) // 128) * 128
    assert u_sub <= 32640, f"triple table too large for int16: {u_sub}"

    mega = np.zeros((NCORES, NGRP, u_sub, C), dtype=np.float16)
    idxw = np.zeros((NCORES, P, NT * TRI_S), dtype=np.int16)
    for c in range(NCORES):
        units_full = np.zeros((9, NT * P), dtype=np.int64)
        for g, (used, units) in enumerate(per_core[c]):
            rows = len(used)
            _, q2, q3 = _units_total(rows)
            X = features[used].astype(np.float16)
            m = mega[c, g]
            m[0:rows] = X
            m[q2 + 5 + 2 * np.arange(rows)] = X
            m[q3 - 4] = X[0]
            m[q3 + 4 * np.arange(rows) + 2] = X
            if rows > 1:
                m[q3 + 4 * np.arange(rows - 1) + 3] = X[1:]
            p0 = g * TILE_GRP * P
            units_full[:, p0:p0 + units.shape[1]] = units
            pend = min((g + 1) * TILE_GRP, NT) * P
            if pend > p0 + units.shape[1]:
                units_full[:, p0 + units.shape[1]:pend] = q2
        ua = units_full.reshape(9, NT, P)
        out = np.zeros((NT, TRI_NIDX), dtype=np.int64)
        for d in range(9):
            out[:, d * P:(d + 1) * P] = ua[d]
        wrapped = out.reshape(NT, TRI_S, 16).transpose(0, 2, 1)
        wrapped = np.tile(wrapped, (1, 8, 1))
        idxw[c] = wrapped.transpose(1, 0, 2).reshape(P, NT * TRI_S)
    return mega, idxw, u_sub


def _build_weight_input(weight):
    w = weight.astype(np.float16).reshape(TAPS, CHUNKS, P, C)
    return np.ascontiguousarray(w.transpose(2, 0, 1, 3).reshape(P, TAPS * CHUNKS * C))


# ------------------------------------------------------------- device kernel --

def _build_bass(u_sub):
    import concourse.bacc as bacc
    import concourse.bass as bass
    import concourse.tile as tile
    from concourse import mybir

    F16, F32, I16 = mybir.dt.float16, mybir.dt.float32, mybir.dt.int16
    nc = bacc.Bacc("TRN2", target_bir_lowering=False, debug=False,
                   num_devices=NCORES, dynamic_dma_scratch_size=65536)
    mega = nc.dram_tensor("mega", [NGRP * u_sub, C], F16, kind="ExternalInput").ap()
    idx = nc.dram_tensor("idx", [P, NT * TRI_S], I16, kind="ExternalInput").ap()
    wts = nc.dram_tensor("wts", [P, TAPS * CHUNKS * C], F16, kind="ExternalInput").ap()
    out = nc.dram_tensor("out", [NLOC, C], F32, kind="ExternalOutput").ap()

    with tile.TileContext(nc) as tc, ExitStack() as ctx:
        const_pool = ctx.enter_context(tc.tile_pool(name="const", bufs=1))
        gpool = ctx.enter_context(tc.tile_pool(name="gather", bufs=3))
        pspool = ctx.enter_context(tc.tile_pool(name="psum", bufs=4, space="PSUM"))
        opool = ctx.enter_context(tc.tile_pool(name="outp", bufs=3))

        w_tile = const_pool.tile([P, TAPS * CHUNKS * C], F16, tag="wts")
        nc.sync.dma_start(out=w_tile[:], in_=wts[:])
        idx_tile = const_pool.tile([P, NT * TRI_S], I16, tag="idx")
        nc.sync.dma_start(out=idx_tile[:], in_=idx[:])

        for t in range(NT):
            g = t // TILE_GRP
            src = bass.AP(mega.tensor, g * u_sub * C, [[C, u_sub - 2], [1, 768]])
            gt = gpool.tile([P, 6, TRI_NIDX], F16, tag="g")
            nc.gpsimd.dma_gather(
                out_ap=gt[:, :, :],
                in_ap=src,
                idxs_ap=idx_tile[:, t * TRI_S:(t + 1) * TRI_S],
                num_idxs=TRI_NIDX,
                num_idxs_reg=TRI_NIDX,
                elem_size=768,
                elem_step=C,
                transpose=True,
                single_packet=False,
            )
            ps = pspool.tile([P, C], F32)
            i_mm = 0
            for d in range(9):
                for dzi in range(3):
                    k = d * 3 + dzi
                    for cc in range(CHUNKS):
                        nc.tensor.matmul(
                            ps[:, :],
                            lhsT=gt[:, dzi * 2 + cc, d * P:(d + 1) * P],
                            rhs=w_tile[:, (k * CHUNKS + cc) * C:(k * CHUNKS + cc + 1) * C],
                            start=(i_mm == 0),
                            stop=(i_mm == TAPS * CHUNKS - 1),
                        )
                        i_mm += 1
            o = opool.tile([P, C], F32)
            nc.vector.tensor_copy(o[:, :], ps[:, :])
            rows = min(P, NLOC - t * P)
            nc.sync.dma_start(out=out[t * P:t * P + rows, :], in_=o[:rows, :])
    nc.compile()
    return nc


# --------------------------------------------------------------- entry point --

def kernel(features, depth, weight):
    from concourse.bass_utils import run_bass_kernel_spmd

    features = np.asarray(features, dtype=np.float32)
    depth = np.asarray(depth, dtype=np.float32)
    weight = np.asarray(weight, dtype=np.float32)

    coord = _compute_coords(depth)
    nid, valid = _compute_nid_valid(coord)
    perm = _core_point_assignment(coord)
    mega, idxw, u_sub = _build_triple_tables(features, coord, nid, valid, perm)
    w_dev = _build_weight_input(weight)

    if u_sub not in _COMPILED:
        _COMPILED[u_sub] = _build_bass(u_sub)
    nc = _COMPILED[u_sub]

    in_maps = [{"mega": mega[c].reshape(-1, C), "idx": idxw[c], "wts": w_dev}
               for c in range(NCORES)]
    res = run_bass_kernel_spmd(nc, in_maps, list(range(NCORES)))

    out = np.empty((N, C), dtype=np.float32)
    for c in range(NCORES):
        out[perm[c]] = res.results[c]["out"]
    return out
